# revision 1
# baseline (speedup 1.0000x reference)
"""DoRA embedding kernel for 8 Trainium2 NeuronCores.

Math (reference):
    C = E + s * A @ B                  # [V, D]
    n = max(||C||_col, 1e-8)           # [D]
    out = (C / n * mag)[token_ids]     # [B, S, D]

Strategy: shard D=768 columns across 8 cores (96 cols each). Each core:
  pass 1: stream its E column-slice (bf16, transposed [96, V]) through PE
          (lora AB matmul + identity-matmul accumulate of E into PSUM),
          square-reduce along vocab on ACT/DVE -> column sumsq. No
          cross-core communication needed (each core owns its columns).
  scale:  rsqrt via ACT sqrt + DVE reciprocal + one Newton step, * mag.
  pass 2: dma_gather of all 16384 tokens from a pair-row table
          ([25152, 256] f32 rows = two vocab rows of E-slice + A packed
          per row, 1024B each) -- pairs because dma_gather indices are
          int16 and V=50257 > 32767; parity select on DVE, A^T via PE
          transpose, A@(s*B) matmul, add, scale, store [16384, 96].
Host: slice/transpose/pad tables per core, gather outputs with one concat.
"""

import sys
from contextlib import ExitStack

import numpy as np

for _p in ("/opt/trn_rl_repo",):
    if _p not in sys.path:
        sys.path.append(_p)

import ml_dtypes
import concourse.bass as bass
import concourse.bacc as bacc
import concourse.tile as tile
from concourse import mybir, bass_utils

F32 = mybir.dt.float32
BF16 = mybir.dt.bfloat16
I16 = mybir.dt.int16
ALU = mybir.AluOpType
ACTF = mybir.ActivationFunctionType

V, D, R = 50257, 768, 16
SCALING = 32.0 / 16.0
N_CORES = 8
CPC = D // N_CORES          # columns per core = 96
T1 = 2048                   # pass-1 macro tile (vocab)
VP = ((V + T1 - 1) // T1) * T1          # 51200
PAIRS = ((V + 1) // 2 + 127) // 128 * 128  # 25216 pair rows (padded)
NTOK = 8 * 2048
CHUNK = 1024                # tokens per gather chunk (dma_gather num_idxs>1024 crashes Q7)
ROWB = 128                  # f32 elems per packed pair-half (96 E + 16 A + pad)
ELEM = 2 * ROWB             # 256 f32 = 1024B per pair row


def _apply_drain_patch():
    """walrus in this container rejects >1 sem-wait on the Tile tail drain
    ("Too many sync wait commands"); split the waits across chained drains."""
    import concourse.tile as _tile_mod
    if getattr(_tile_mod.TileContext, "_drain_patch_applied", False):
        return

    def _drain_and_barrier(self, tick_clock, wait_clock):
        from concourse.tile import ScopedClock

        nc = self.nc
        drain_inst = nc.sync.drain()
        wait_clock.add_sem_waits(
            drain_inst.ins, ScopedClock({None: tick_clock.global_clock})
        )
        si = drain_inst.ins.sync_info
        if si is not None and si.on_wait and len(si.on_wait) > 1:
            waits = list(si.on_wait)
            del si.on_wait[1:]
            for w in waits[1:]:
                extra = nc.sync.drain()
                esi = extra.ins.sync_info
                if esi is None:
                    extra.ins.sync_info = mybir.SyncInfo(on_wait=[w], on_update=[])
                else:
                    esi.on_wait.append(w)
        nc.all_engine_barrier()
        assert self.sems is not None
        popped = nc._tile_sem_poison_stack.pop()
        assert popped is self._sem_poison
        nc.clear_and_free_semaphores(list(self.sems.allocated().values()))
        nc.all_engine_barrier()

    _tile_mod.TileContext._drain_and_barrier = _drain_and_barrier
    _tile_mod.TileContext._drain_patch_applied = True


def _bcast(ap_2d, n):
    """[P, W] AP -> [P, W, n] AP broadcasting along a new 0-stride inner dim."""
    return bass.AP(
        tensor=ap_2d.tensor,
        offset=ap_2d.offset,
        ap=[list(ap_2d.ap[0]), list(ap_2d.ap[1]), [0, n]],
    )


_CACHED = {}
NREP = 1
VARIANT = "full"  # full | p1 | p2


def _build():
    key = (NREP, VARIANT, CHUNK)
    if key in _CACHED:
        return _CACHED[key]
    _apply_drain_patch()

    T1L = 1024
    NT = VP // T1L
    CH = NTOK // CHUNK
    NSLOT = CHUNK // 128

    nc = bacc.Bacc("TRN2", target_bir_lowering=False, debug=False)
    d_ecolT = nc.dram_tensor("ecolT", [CPC, VP], BF16, kind="ExternalInput").ap()
    d_at = nc.dram_tensor("a_t", [R, VP], BF16, kind="ExternalInput").ap()
    d_bsb = nc.dram_tensor("bs_bf", [R, CPC], BF16, kind="ExternalInput").ap()
    d_bsf = nc.dram_tensor("bs_f32", [R, CPC], F32, kind="ExternalInput").ap()
    d_mag = nc.dram_tensor("magT", [1, CPC], F32, kind="ExternalInput").ap()
    d_etab = nc.dram_tensor("etab", [PAIRS, ELEM], F32, kind="ExternalInput").ap()
    d_pidx = nc.dram_tensor("pidx", [128, NTOK // 16], I16, kind="ExternalInput").ap()
    d_m1 = nc.dram_tensor("m1", [128, CH * NSLOT], F32, kind="ExternalInput").ap()
    d_idf = nc.dram_tensor("identf", [128, 128], F32, kind="ExternalInput").ap()
    d_idb = nc.dram_tensor("identb", [CPC, CPC], BF16, kind="ExternalInput").ap()
    d_ones = nc.dram_tensor("ones1", [1, 128], F32, kind="ExternalInput").ap()
    d_out = nc.dram_tensor("out", [NTOK, CPC], F32, kind="ExternalOutput").ap()

    with tile.TileContext(nc) as tc, ExitStack() as ctx:
        const = ctx.enter_context(tc.tile_pool(name="const", bufs=1))

        at_sb = const.tile([R, VP], BF16)
        nc.sync.dma_start(out=at_sb, in_=d_at)
        bsb_sb = const.tile([R, CPC], BF16)
        nc.sync.dma_start(out=bsb_sb, in_=d_bsb)
        bsf_sb = const.tile([R, CPC], F32)
        nc.sync.dma_start(out=bsf_sb, in_=d_bsf)
        mag_sb = const.tile([1, CPC], F32)
        nc.sync.dma_start(out=mag_sb, in_=d_mag)
        idf_sb = const.tile([128, 128], F32)
        nc.sync.dma_start(out=idf_sb, in_=d_idf)
        idb_sb = const.tile([CPC, CPC], BF16)
        nc.sync.dma_start(out=idb_sb, in_=d_idb)
        ones_sb = const.tile([1, 128], F32)
        nc.sync.dma_start(out=ones_sb, in_=d_ones)
        m1_sb = const.tile([128, CH * NSLOT], F32)
        nc.sync.dma_start(out=m1_sb, in_=d_m1)
        pidx_sb = const.tile([128, NTOK // 16], I16)
        nc.sync.dma_start(out=pidx_sb, in_=d_pidx)

        def _emit(rep_pool, p1e, p1ps, p1s, sc, p2g, p2s, p2tp, p2ab, p2o):
            acc_sb = rep_pool.tile([CPC, NT], F32)

            # ---- pass 1: column sumsq over vocab ----
            if VARIANT in ("p2", "p2g", "p2n"):
                scale_bc = rep_pool.tile([128, CPC], F32)
                nc.vector.memset(scale_bc, 1.0)
            else:
                for i in range(NT):
                    et = p1e.tile([CPC, T1L], BF16)
                    nc.sync.dma_start(
                        out=et, in_=d_ecolT[:, i * T1L : (i + 1) * T1L]
                    )
                    ps = p1ps.tile([CPC, T1L], F32, tag="ps")
                    use_act = True
                    for j in range(T1L // 512):
                        sl = slice(j * 512, (j + 1) * 512)
                        nc.tensor.matmul(
                            ps[:, sl],
                            bsb_sb[:, :],
                            at_sb[:, i * T1L + j * 512 : i * T1L + (j + 1) * 512],
                            start=True,
                            stop=not use_act,
                        )
                        if use_act:
                            nc.tensor.matmul(
                                ps[:, sl], idb_sb[:, :], et[:, sl],
                                start=False, stop=True,
                            )
                    if use_act:
                        sq = p1s.tile([CPC, T1L], BF16, tag="sq")
                        nc.scalar.activation(
                            sq, ps, ACTF.Square, accum_out=acc_sb[:, i : i + 1]
                        )
                    else:
                        comb = p1s.tile([CPC, T1L], BF16, tag="comb")
                        nc.vector.tensor_tensor(
                            out=comb, in0=ps, in1=et, op=ALU.add
                        )
                        sq = p1s.tile([CPC, T1L], BF16, tag="sq")
                        nc.vector.scalar_tensor_tensor(
                            out=sq, in0=comb, scalar=1.0, in1=comb,
                            op0=ALU.bypass, op1=ALU.mult,
                            accum_out=acc_sb[:, i : i + 1],
                        )

                # ---- scale vector ----
                ss = sc.tile([CPC, 1], F32)
                nc.vector.reduce_sum(out=ss, in_=acc_sb, axis=mybir.AxisListType.X)
                ssT_ps = p1ps.tile([1, CPC], F32, tag="ps")
                nc.tensor.transpose(ssT_ps, ss, idf_sb[:CPC, :CPC])
                ssT = sc.tile([1, CPC], F32)
                nc.vector.tensor_copy(out=ssT, in_=ssT_ps)
                nrm = sc.tile([1, CPC], F32)
                nc.scalar.activation(nrm, ssT, ACTF.Sqrt)
                nrm2 = sc.tile([1, CPC], F32)
                nc.vector.tensor_scalar(
                    out=nrm2, in0=nrm, scalar1=1e-8, scalar2=None, op0=ALU.max
                )
                r0 = sc.tile([1, CPC], F32)
                nc.vector.reciprocal(out=r0, in_=nrm2)
                t = sc.tile([1, CPC], F32)
                nc.vector.tensor_tensor(out=t, in0=ssT, in1=r0, op=ALU.mult)
                nc.vector.tensor_tensor(out=t, in0=t, in1=r0, op=ALU.mult)
                nc.vector.tensor_scalar(
                    out=t, in0=t, scalar1=-0.5, scalar2=1.5,
                    op0=ALU.mult, op1=ALU.add,
                )
                r1 = sc.tile([1, CPC], F32)
                nc.vector.tensor_tensor(out=r1, in0=r0, in1=t, op=ALU.mult)
                scl = sc.tile([1, CPC], F32)
                nc.vector.tensor_tensor(out=scl, in0=r1, in1=mag_sb, op=ALU.mult)
                bc_ps = p1ps.tile([128, CPC], F32, tag="ps")
                nc.tensor.matmul(
                    bc_ps, ones_sb[:, :], scl[:, :], start=True, stop=True
                )
                scale_bc = rep_pool.tile([128, CPC], F32)
                nc.vector.tensor_copy(out=scale_bc, in_=bc_ps)

            # ---- pass 2: gather + lora + scale ----
            if VARIANT == "p1":
                outt0 = rep_pool.tile([128, CPC], F32)
                nc.vector.tensor_copy(out=outt0, in_=scale_bc)
                nc.sync.dma_start(out=d_out[0:128, :], in_=outt0)
                return
            for c in range(CH):
                eg = p2g.tile([128, NSLOT, ELEM], F32)
                if VARIANT == "p2n":
                    nc.vector.memset(eg, 0.25)
                else:
                    nc.gpsimd.dma_gather(
                        eg[:, :, :],
                        d_etab[:, :],
                        pidx_sb[:, c * (CHUNK // 16) : (c + 1) * (CHUNK // 16)],
                        num_idxs=CHUNK,
                        num_idxs_reg=CHUNK,
                        elem_size=ELEM,
                    )
                if VARIANT == "p2g":
                    continue
                lo = eg[:, :, 0 : CPC + R]
                hi = eg[:, :, ROWB : ROWB + CPC + R]
                mslice = m1_sb[:, c * NSLOT : (c + 1) * NSLOT]
                dd = p2s.tile([128, NSLOT, CPC + R], F32, tag="dd")
                nc.vector.tensor_tensor(out=dd, in0=hi, in1=lo, op=ALU.subtract)
                nc.vector.tensor_tensor(
                    out=dd, in0=dd, in1=_bcast(mslice, CPC + R), op=ALU.mult
                )
                sel = p2s.tile([128, NSLOT, CPC + R], F32, tag="sel")
                nc.vector.tensor_tensor(out=sel, in0=lo, in1=dd, op=ALU.add)

                tp_ps = p2tp.tile([R, NSLOT, 128], F32)
                for s in range(NSLOT):
                    nc.tensor.transpose(
                        tp_ps[:, s, :], sel[:, s, CPC : CPC + R], idf_sb[:, :]
                    )
                astp = p2s.tile([R, NSLOT, 128], F32, tag="astp")
                nc.scalar.copy(out=astp, in_=tp_ps)

                ab_ps = p2ab.tile([128, NSLOT, 128], F32)
                for s in range(NSLOT):
                    nc.tensor.matmul(
                        ab_ps[:, s, 0:CPC],
                        astp[:, s, :],
                        bsf_sb[:, :],
                        start=True,
                        stop=False,
                    )
                    # += E_sel via identity matmul (keeps the add off DVE)
                    nc.tensor.matmul(
                        ab_ps[:, s, 0:CPC],
                        idf_sb[:, :],
                        sel[:, s, 0:CPC],
                        start=False,
                        stop=True,
                    )

                outt = p2o.tile([128, NSLOT, CPC], F32)
                nc.vector.tensor_tensor(
                    out=outt, in0=ab_ps[:, :, 0:CPC],
                    in1=_bcast2(scale_bc, NSLOT), op=ALU.mult
                )
                dview = d_out[c * CHUNK : (c + 1) * CHUNK, :].rearrange(
                    "(s p) f -> p s f", p=128
                )
                nc.sync.dma_start(out=dview, in_=outt)

        for _rep in range(NREP):
            if _rep:
                tc.strict_bb_all_engine_barrier()
            with (
                tc.tile_pool(name=f"rep{_rep}", bufs=1) as rep_pool,
                tc.tile_pool(name=f"p1e{_rep}", bufs=3) as p1e,
                tc.tile_pool(name=f"p1ps{_rep}", bufs=2, space="PSUM") as p1ps,
                tc.tile_pool(name=f"p1s{_rep}", bufs=2) as p1s,
                tc.tile_pool(name=f"sc{_rep}", bufs=1) as sc,
                tc.tile_pool(name=f"p2g{_rep}", bufs=4) as p2g,
                tc.tile_pool(name=f"p2s{_rep}", bufs=2) as p2s,
                tc.tile_pool(name=f"p2tp{_rep}", bufs=1, space="PSUM") as p2tp,
                tc.tile_pool(name=f"p2ab{_rep}", bufs=1, space="PSUM") as p2ab,
                tc.tile_pool(name=f"p2o{_rep}", bufs=3) as p2o,
            ):
                _emit(rep_pool, p1e, p1ps, p1s, sc, p2g, p2s, p2tp, p2ab, p2o)

    nc.compile()
    _CACHED[key] = nc
    return nc


def _bcast2(ap_2d, n):
    """[P, W] AP -> [P, n, W] AP broadcasting along a new 0-stride middle dim."""
    return bass.AP(
        tensor=ap_2d.tensor,
        offset=ap_2d.offset,
        ap=[list(ap_2d.ap[0]), [0, n], list(ap_2d.ap[1])],
    )


def _host_prep(inputs, embeddings, lora_a, lora_b, magnitude):
    E = np.asarray(embeddings, np.float32)
    A = np.asarray(lora_a, np.float32)
    B = np.asarray(lora_b, np.float32)
    mag = np.asarray(magnitude, np.float32)
    ids = np.asarray(inputs).astype(np.int64).reshape(-1)
    CH = NTOK // CHUNK

    Epad = np.zeros((2 * PAIRS, D), np.float32)
    Epad[:V] = E
    Apad = np.zeros((2 * PAIRS, R), np.float32)
    Apad[:V] = A

    at_np = np.zeros((R, VP), dtype=ml_dtypes.bfloat16)
    at_np[:, :V] = A.T.astype(ml_dtypes.bfloat16)

    pidx_np = np.tile(
        (ids // 2).astype(np.int16).reshape(NTOK // 16, 16).T, (8, 1)
    ).copy()
    ids3 = ids.reshape(CH, CHUNK // 128, 128)
    m1_np = np.ascontiguousarray(
        (ids3 % 2).astype(np.float32).transpose(2, 0, 1).reshape(128, CH * (CHUNK // 128))
    )

    idf_np = np.eye(128, dtype=np.float32)
    idb_np = np.eye(CPC, dtype=ml_dtypes.bfloat16)
    ones_np = np.ones((1, 128), np.float32)

    in_maps = []
    for c in range(N_CORES):
        cols = slice(CPC * c, CPC * (c + 1))
        ecolT = np.zeros((CPC, VP), dtype=ml_dtypes.bfloat16)
        ecolT[:, :V] = E[:, cols].T.astype(ml_dtypes.bfloat16)
        ec = Epad[:, cols]
        etab = np.zeros((PAIRS, ELEM), np.float32)
        etab[:, 0:CPC] = ec[0::2]
        etab[:, CPC : CPC + R] = Apad[0::2]
        etab[:, ROWB : ROWB + CPC] = ec[1::2]
        etab[:, ROWB + CPC : ROWB + CPC + R] = Apad[1::2]
        bs = SCALING * B[:, cols]
        in_maps.append(
            {
                "ecolT": ecolT,
                "a_t": at_np,
                "bs_bf": bs.astype(ml_dtypes.bfloat16),
                "bs_f32": np.ascontiguousarray(bs),
                "magT": np.ascontiguousarray(mag[cols])[None, :],
                "etab": etab,
                "pidx": pidx_np,
                "m1": m1_np,
                "identf": idf_np,
                "identb": idb_np,
                "ones1": ones_np,
            }
        )
    return in_maps


def kernel(inputs, embeddings, lora_a, lora_b, magnitude, _trace=False):
    nc = _build()
    in_maps = _host_prep(inputs, embeddings, lora_a, lora_b, magnitude)
    res = bass_utils.run_bass_kernel_spmd(
        nc, in_maps, core_ids=list(range(N_CORES)), trace=_trace
    )
    out = np.concatenate([res.results[c]["out"] for c in range(N_CORES)], axis=1)
    out = out.reshape(np.asarray(inputs).shape + (D,))
    if _trace:
        return out, res
    return out

